# revision 73
# baseline (speedup 1.0000x reference)
"""Multi-head attention (B=4, N=2048, d_model=1024, 16 heads) on 8 trn2 cores.

Sharding: data-parallel over batch (4) x Megatron tensor-parallel over heads
(2-way column-split Wq/Wk/Wv, row-split Wo).  Core c handles batch c//2 and
heads [8*(c%2), 8*(c%2)+8).  Each core emits a partial Y^T [1024, 2048] in
bf16; the host sums core pairs, transposes, and adds the output bias.  No
on-device collectives (a 2-rank AllReduce costs more than the whole compute).

On-device pipeline per core (bf16 matmuls, fp32 PSUM accumulate):
  Q^T [512,2048], zero-padded K^T (each head's 64 rows padded to a full
  128-partition stationary so score LDWEIGHTS background-load instead of
  stalling like row-tiled ones), and V [2048, 8x(64+1)] with a ones column
  so the softmax denominators fall out of the AV matmul.  Per head pair and
  kv-tile: S^T = K_h @ Q_h^T (two serial full-width matmuls), exp on the
  scalar engine with the 1/sqrt(64) scale folded in, then a triu-mask
  multiply on GpSimd for diagonal tiles only — scores/exp/AV all restrict to
  the un-masked column range (q >= kv) of diagonal tiles.  AV (lhsT = V_aug)
  accumulates A^T plus the denominator row; 1/sums are batched bf16
  reciprocals broadcast over the pair's depth rows by a selector matmul.
  Y^T = WoT^T @ A^T, stored bf16.

Scheduling: engines run in-order, so everything is software-pipelined by
emission order.  xT arrives in per-k chunks over two DMA queues and the
projection contraction order chases the arrivals.  AV runs TWO kv-steps
behind scores so the scores->exp->mask chain (~2us) hides entirely.  Next
pair's projections, previous pair's normalization, and the finished
t-blocks of the output projection are paced into the attention loop as PE
filler; the final t-block's output projection is split f=0..2 / f=3 and
interleaved (with warm-keeper matmuls) into the tail reciprocal chain so
the HAM clock gate never re-throttles the PE.
"""

import sys

for _p in ("/opt/trn_rl_repo",):
    if _p not in sys.path:
        sys.path.insert(0, _p)

from contextlib import ExitStack

import ml_dtypes
import numpy as np

import concourse.bass as bass
import concourse.mybir as mybir
import concourse.tile as tile_mod
from concourse.vector_clock import ScopedClock

# ---------------------------------------------------------------------------
# Workaround: this walrus build rejects >1 sync wait on a Drain (CTRL_NO)
# instruction ("Too many sync wait commands").  Tile's end-of-context drain
# carries one wait per live processor, so redistribute the extras onto
# individual EventSemaphore wait instructions.
# ---------------------------------------------------------------------------


def _patched_drain_and_barrier(self, tick_clock, wait_clock):
    nc = self.nc
    drain_inst = nc.sync.drain()
    wait_clock.add_sem_waits(
        drain_inst.ins, ScopedClock({None: tick_clock.global_clock})
    )
    si = drain_inst.ins.sync_info
    waits = list(si.on_wait) if si is not None else []
    if len(waits) > 1:
        assert self.sems is not None
        num2handle = {h.num: h for h in self.sems.allocated().values()}
        drain_inst.ins.sync_info = mybir.SyncInfo(
            on_wait=[waits[0]], on_update=list(si.on_update)
        )
        for w in waits[1:]:
            h = num2handle.get(w.id)
            assert h is not None, f"no sem handle for {w.ant_name} (id {w.id})"
            assert w.wait_mode.startswith("sem-ge"), w.wait_mode
            nc.sync.wait_ge(h, w.wait_value)

    nc.all_engine_barrier()
    assert self.sems is not None
    popped = nc._tile_sem_poison_stack.pop()
    assert popped is self._sem_poison
    nc.clear_and_free_semaphores(list(self.sems.allocated().values()))
    nc.all_engine_barrier()


tile_mod.TileContext._drain_and_barrier = _patched_drain_and_barrier


def _spill_excess_waits(nc: bass.Bass) -> None:
    """This walrus build accepts at most 1 sync wait per instruction (2 for
    EventSemaphore).  Move excess waits onto EventSemaphore instructions
    inserted just before the over-subscribed instruction on the same engine."""
    n_new = 0
    for f in nc.m.functions:
        for blk in f.blocks:
            il = blk.instructions
            out = []
            changed = False
            for inst in il:
                si = inst.sync_info
                waits = list(si.on_wait) if si is not None else []
                cap = 2 if isinstance(inst, mybir.InstEventSemaphore) else 1
                if len(waits) > cap:
                    changed = True
                    extra, keep = waits[:-cap], waits[-cap:]
                    inst.sync_info = mybir.SyncInfo(
                        on_wait=keep, on_update=list(si.on_update)
                    )
                    for j in range(0, len(extra), 2):
                        n_new += 1
                        out.append(
                            mybir.InstEventSemaphore(
                                name=f"{inst.name}-xw{j}",
                                ins=[],
                                outs=[],
                                engine=inst.engine,
                                sync_info=mybir.SyncInfo(
                                    on_wait=extra[j:j + 2], on_update=[]
                                ),
                            )
                        )
                out.append(inst)
            if changed:
                il[:] = out

# ---------------------------------------------------------------------------
# Problem shapes (hardcoded per the task contract).
# ---------------------------------------------------------------------------
B, N, D = 4, 2048, 1024
NHEAD, DEPTH = 16, 64
NCORES = 8
FH = 512          # features per core (8 heads x 64)
HPC = 8           # heads per core
P = 128           # SBUF partitions
TB = 512          # token block (matmul moving free dim)
NTB = N // TB     # 4 token blocks
KT = D // P       # 8 contraction tiles for the projections
NFT = FH // P     # 4 feature tiles (= head pairs)
NTT = N // P      # 16 token tiles
NKV = N // P      # 16 kv tiles
SCALE = 1.0 / np.sqrt(DEPTH)
F32, F32R, BF16 = mybir.dt.float32, mybir.dt.float32r, mybir.dt.bfloat16

_BF16 = ml_dtypes.bfloat16


def build_program(variant: str) -> bass.Bass:
    """variant: 'causal' (tril mask), 'full' (all-true mask), 'general'."""
    assert variant in ("causal", "full", "general")
    nc = bass.Bass()

    # pre-tiled on the host: partition-major layouts for fast (contiguous
    # per-partition) DMA
    xT = nc.declare_dram_parameter("xT", [P, KT, N], BF16, isOutput=False)
    wqT = nc.declare_dram_parameter("wqT", [NFT, P, KT, P], BF16, isOutput=False)
    wkT = nc.declare_dram_parameter("wkT", [NFT, P, KT, P], BF16, isOutput=False)
    wvT = nc.declare_dram_parameter("wvT", [P, KT, FH], BF16, isOutput=False)
    woT = nc.declare_dram_parameter("woT", [P, NFT, D], BF16, isOutput=False)
    bq2 = nc.declare_dram_parameter("bq2", [P, NFT], F32, isOutput=False)
    bk2 = nc.declare_dram_parameter("bk2", [P, NFT], F32, isOutput=False)
    # V bias replicated across partitions: added during the DVE evacuation
    # of the V projection instead of via an extra PE matmul
    bvf = nc.declare_dram_parameter("bvf", [P, FH], BF16, isOutput=False)
    # block-diagonal selector for broadcasting 1/sums rows (bf16)
    sel_d = nc.declare_dram_parameter("sel", [8, 8 * DEPTH], BF16, isOutput=False)
    if variant == "causal":
        # the within-tile triangle: mb[p, c] = 1 iff c >= p
        mb = nc.declare_dram_parameter("mb", [P, P], BF16, isOutput=False)
    elif variant == "general":
        mb = nc.declare_dram_parameter("mb", [P, NKV, NTB, TB], BF16, isOutput=False)
    yT = nc.declare_dram_parameter("yT", [D, N], BF16, isOutput=True)

    def nkv_of(qb):
        return 4 * (qb + 1) if variant == "causal" else NKV

    with tile_mod.TileContext(nc) as tc, ExitStack() as ctx:
        res = ctx.enter_context(tc.tile_pool(name="res", bufs=1))
        wp = ctx.enter_context(tc.tile_pool(name="w", bufs=2))
        pp = ctx.enter_context(tc.tile_pool(name="ppair", bufs=8))
        sums = ctx.enter_context(tc.tile_pool(name="sums", bufs=4))
        yst = ctx.enter_context(tc.tile_pool(name="yst", bufs=4))
        # PSUM: shared accumulator tag (2 banks) + score pair tiles (4) +
        # the two AV accumulators (2) = 8 banks exactly.
        mmp = ctx.enter_context(tc.tile_pool(name="mmp", bufs=2, space="PSUM"))
        pssp = ctx.enter_context(tc.tile_pool(name="pssp", bufs=2, space="PSUM"))
        psav = ctx.enter_context(tc.tile_pool(name="psav", bufs=1, space="PSUM"))
        if variant == "general":
            mgp = ctx.enter_context(tc.tile_pool(name="mg", bufs=4))

        bq_sb = res.tile([P, NFT], F32)
        bk_sb = res.tile([P, NFT], F32)
        bv_sb = res.tile([P, FH], BF16)
        sel_sb = res.tile([8, 8 * DEPTH], BF16)
        mb_sb = None
        if variant == "causal":
            mb_sb = res.tile([P, P], BF16, name="mb_sb")

        def fetch_biases():
            # tiny, but gate the first bias-adds (psum evacuation) — issued
            # before the weight fetches
            nc.scalar.dma_start(bq_sb[:], bq2[:])
            nc.scalar.dma_start(bk_sb[:], bk2[:])

        def fetch_residents():
            # needed only once attention starts (~40us in)
            nc.scalar.dma_start(bv_sb[:], bvf[:])
            nc.scalar.dma_start(sel_sb[:], sel_d[:])
            if variant == "causal":
                nc.scalar.dma_start(mb_sb[:], mb[:])

        qt_sb = res.tile([P, NFT, N], BF16)   # Q^T  [feat, tok]
        # K^T stored zero-padded per head: slot hh holds the head's 64 rows
        # in its own partition half and zeros in the other, so score matmuls
        # are full-width non-tiled (their LDWEIGHTS background-load instead
        # of stalling behind the in-flight matmul like row-tiled ones do).
        kt_sb = res.tile([P, NFT, 2, N], BF16)
        nc.vector.memset(kt_sb[DEPTH:P, :, 0, :], 0.0)
        nc.vector.memset(kt_sb[0:DEPTH, :, 1, :], 0.0)
        v_sb = res.tile([P, NTT, HPC, DEPTH + 1], BF16)  # V + ones col
        nc.gpsimd.memset(v_sb[:, :, :, DEPTH], 1.0)
        a_sb = res.tile([P, NFT, N], BF16)    # A^T (attention output)

        xt_sb = res.tile([P, KT, N], BF16)
        xT3 = xT
        wv_sb = res.tile([P, KT, FH], BF16)

        wq_sbs, wk_sbs = {}, {}

        def fetch_w(ft, eng=None):
            eng = eng or nc.sync
            wq_sbs[ft] = wp.tile([P, KT, P], BF16, tag="wq", name="wq_sb")
            wk_sbs[ft] = wp.tile([P, KT, P], BF16, tag="wk", name="wk_sb")
            eng.dma_start(wq_sbs[ft][:], wqT[ft])
            eng.dma_start(wk_sbs[ft][:], wkT[ft])

        # contraction order chases the startup DMA arrival order (sync queue
        # carries k0-3, gpsimd k4-7, both draining in parallel)
        KORD = (0, 4, 1, 5, 2, 6, 3, 7)

        def emit_qk_group(ft, tb, which):
            ts = slice(tb * TB, (tb + 1) * TB)
            w_sb = wq_sbs[ft] if which == "q" else wk_sbs[ft]
            dst = qt_sb if which == "q" else kt_sb
            bias = bq_sb if which == "q" else bk_sb
            ps = mmp.tile([P, TB], F32, tag="acc", name="pqk")
            for j, k in enumerate(KORD):
                nc.tensor.matmul(
                    ps[:], w_sb[:, k, :], xt_sb[:, k, ts],
                    start=(j == 0), stop=(j == KT - 1),
                )
            if which == "k":
                for hh in (0, 1):
                    rows = slice(DEPTH * hh, DEPTH * hh + DEPTH)
                    nc.vector.tensor_tensor(
                        kt_sb[rows, ft, hh, ts], ps[rows],
                        bias[rows, ft, None].to_broadcast((DEPTH, TB)),
                        mybir.AluOpType.add,
                    )
            else:
                nc.vector.tensor_tensor(
                    dst[:, ft, ts], ps[:],
                    bias[:, ft, None].to_broadcast((P, TB)),
                    mybir.AluOpType.add,
                )

        def proj_filler(ft):
            for tb in range(NTB):
                for which in ("q", "k"):
                    yield lambda ft=ft, tb=tb, which=which: emit_qk_group(
                        ft, tb, which
                    )

        def emit_v_group(tt):
            pv = mmp.tile([P, TB], F32, tag="acc", name="pv")
            for j, k in enumerate(KORD):
                nc.tensor.matmul(
                    pv[:], xt_sb[:, k, tt * P:(tt + 1) * P], wv_sb[:, k, :],
                    start=(j == 0), stop=(j == KT - 1),
                )
            nc.vector.tensor_tensor(
                v_sb[:, tt, :, 0:DEPTH],
                pv[:].rearrange("p (h d) -> p h d", h=HPC),
                bv_sb[:].rearrange("p (h d) -> p h d", h=HPC),
                mybir.AluOpType.add,
            )

        sums_ps = {}
        rall_store = {}

        def emit_norm_qb(pr, rall, row0, qb, tail=False):
            """bc = broadcast of 1/sums rows (row0, row0+1) over the head
            depth; one in-place multiply normalizes both heads of the pair.
            The tail variant borrows a retired AV psum bank so the held
            output-projection partials can't deadlock the acc ring."""
            qs = slice(qb * TB, (qb + 1) * TB)
            nrows = rall.shape[0]
            if tail:
                bc = psav.tile([P, TB], F32, tag="av0", name="bc")
            else:
                bc = mmp.tile([P, TB], F32, tag="acc", name="bc")
            nc.tensor.matmul(
                bc[:],
                sel_sb[0:nrows, row0 * DEPTH:(row0 + 2) * DEPTH],
                rall[:],
                start=True, stop=True,
            )
            nc.vector.tensor_tensor(
                a_sb[:, pr, qs], a_sb[:, pr, qs], bc[:],
                mybir.AluOpType.mult,
            )

        def norm_filler(pr):
            def recip(pr=pr, part=0):
                if part == 0:
                    rall_store[pr] = sums.tile([8, TB], BF16, tag="rall", name="rall")
                with nc.allow_low_precision(
                    reason="bf16 1/sum scales each softmax row uniformly"
                ):
                    cs = slice(part * P, (part + 1) * P)
                    nc.vector.reciprocal(
                        rall_store[pr][:, cs], sums_ps[pr][:, cs]
                    )

            for part in range(TB // P):
                yield lambda pr=pr, part=part: recip(pr, part)
            for qb in range(NTB):
                yield lambda pr=pr, qb=qb: emit_norm_qb(
                    pr, rall_store[pr], 2 * qb, qb
                )

        wo_sb = res.tile([P, NFT, D], BF16)
        yT3 = yT.rearrange("(o p) t -> p o t", p=P)

        def outproj_pre(ot, tb):
            """f=0..2 partial accumulation — consumable before the final
            pr's normalization lands, keeping the PE warm through the
            tail reciprocal chain."""
            ts = slice(tb * TB, (tb + 1) * TB)
            py = mmp.tile([P, TB], F32, tag="acc", name="py")
            for f in range(NFT - 1):
                nc.tensor.matmul(
                    py[:], wo_sb[:, f, ot * P:(ot + 1) * P],
                    a_sb[:, f, ts],
                    start=(f == 0), stop=False,
                )
            return py

        def outproj_fin(py, ot, tb):
            ts = slice(tb * TB, (tb + 1) * TB)
            nc.tensor.matmul(
                py[:], wo_sb[:, NFT - 1, ot * P:(ot + 1) * P],
                a_sb[:, NFT - 1, ts],
                start=False, stop=True,
            )
            yt = yst.tile([P, TB], BF16, tag="yt")
            nc.vector.tensor_copy(yt[:], py[:])
            # spread output stores over DMA queues; the scalar (exp) queue
            # only joins for the final q-block, after the last activation
            if tb == NTB - 1:
                eng = (nc.sync, nc.gpsimd, nc.scalar)[ot % 3]
            else:
                eng = nc.sync if ot % 2 == 0 else nc.gpsimd
            eng.dma_start(yT3[:, ot, ts], yt[:])

        def emit_outproj_group(ot, tb):
            outproj_fin(outproj_pre(ot, tb), ot, tb)

        def attention(pr, early, spread, qb_prologue=None):
            """early: fillers consumed one per kv-step (from step 4);
            spread: fillers paced over ~70% of the kv-steps.  Scores run one
            step ahead of AV across qb boundaries so the PE never drains."""
            last = pr == NFT - 1
            if not last:
                sums_p = sums.tile([8, TB], F32, tag="sums_p", name="sums_p")
                sums_ps[pr] = sums_p
            steps = [(qb, kv) for qb in range(NTB) for kv in range(nkv_of(qb))]
            n_slots = len(steps)
            si = 0
            kvstep = 0
            avs = {}
            s3s = {}

            def qb_end(qb):
                qs = slice(qb * TB, (qb + 1) * TB)
                av = avs.pop(qb)
                for hh in (0, 1):
                    srow = sums.tile([P, TB], F32, tag="srow", name="srow")
                    nc.vector.tensor_copy(
                        srow[DEPTH:DEPTH + 1, :], av[hh][DEPTH:DEPTH + 1, :]
                    )
                    if last:
                        nc.gpsimd.dma_start(
                            s3s[qb][hh:hh + 1, :], srow[DEPTH:DEPTH + 1, :]
                        )
                    else:
                        nc.gpsimd.dma_start(
                            sums_p[2 * qb + hh:2 * qb + hh + 1, :],
                            srow[DEPTH:DEPTH + 1, :],
                        )
                    nc.vector.tensor_copy(
                        a_sb[64 * hh:64 * hh + 64, pr, qs], av[hh][0:DEPTH, :]
                    )
                if last:
                    s3 = s3s.pop(qb)

                    rall3 = [None]

                    def norm3_part(part, qb=qb, s3=s3, rall3=rall3):
                        if part == 0:
                            rall3[0] = sums.tile([2, TB], BF16, tag="r3", name="rall3")
                        with nc.allow_low_precision(
                            reason="bf16 1/sum scales each softmax row uniformly"
                        ):
                            cs = slice(part * P, (part + 1) * P)
                            nc.vector.reciprocal(rall3[0][:, cs], s3[:, cs])
                        if part == TB // P - 1:
                            emit_norm_qb(pr, rall3[0], 0, qb, tail=(qb == NTB - 1))

                    if qb == NTB - 1:
                        # final q-block: interleave the f=0..2 output-proj
                        # partials (valid pre-norm) with the reciprocal
                        # chain so the PE never idles into a HAM rethrottle,
                        # then finish each column group after the norm
                        pys = {}

                        def pre(ot, qb=qb):
                            pys[ot] = outproj_pre(ot, qb)

                        def fin(ot, qb=qb):
                            outproj_fin(pys.pop(ot), ot, qb)

                        def warmer(qb=qb):
                            # keeps the PE streaming through the reciprocal
                            # chain so HAM doesn't rethrottle before the
                            # final output-projection burst; output unused
                            dj = psav.tile([P, TB], F32, tag="av1", name="dj")
                            ts_ = slice(qb * TB, (qb + 1) * TB)
                            for f in range(NFT - 1):
                                nc.tensor.matmul(
                                    dj[:], wo_sb[:, f, 0:P], a_sb[:, f, ts_],
                                    start=(f == 0), stop=(f == NFT - 2),
                                )

                        deferred.append(lambda: pre(0))
                        deferred.append(lambda: pre(1))
                        deferred.append(lambda: norm3_part(0))
                        deferred.append(lambda: norm3_part(1))
                        deferred.append(warmer)
                        deferred.append(lambda: norm3_part(2))
                        deferred.append(warmer)
                        deferred.append(lambda: norm3_part(3))
                        deferred.append(warmer)
                        for ot in range(D // P):
                            deferred.append(lambda ot=ot: fin(ot))
                            if ot + 2 < D // P:
                                deferred.append(lambda ot=ot: pre(ot + 2))
                    else:
                        for part in range(TB // P):
                            deferred.append(lambda part=part: norm3_part(part))
                        for ot in range(D // P):
                            deferred.append(
                                lambda ot=ot, tb=qb: emit_outproj_group(ot, tb)
                            )

            def do_av(pqb, pkv, pq0, ppt):
                if pkv == 0:
                    avs[pqb] = [
                        psav.tile([P, TB], F32, tag=f"av{h}", name=f"av{h}")
                        for h in (0, 1)
                    ]
                for hh in (0, 1):
                    nc.tensor.matmul(
                        avs[pqb][hh][0:DEPTH + 1, pq0:TB],
                        v_sb[:, pkv, 2 * pr + hh, :],
                        ppt[:, hh, pq0:TB],
                        start=(pkv == 0), stop=(pkv == nkv_of(pqb) - 1),
                    )
                if pkv == nkv_of(pqb) - 1:
                    qb_end(pqb)

            pendq = []
            for qb, kv in steps:
                if kv == 0 and last:
                    s3s[qb] = sums.tile([2, TB], F32, tag="s3", name="s3")
                # diagonal kv-tiles only need q >= kv: columns < q0 of this
                # q-block are fully masked, so scores/exp/mask/AV all skip
                # them (the per-tile mask reduces to one triu pattern).
                diag = variant == "causal" and kv >= 4 * qb
                q0 = (kv - 4 * qb) * P if diag else 0
                qs = slice(qb * TB + q0, (qb + 1) * TB)
                sp = pssp.tile([P, 2, TB], F32, tag="sp")
                for hh in (0, 1):
                    nc.tensor.matmul(
                        sp[:, hh, q0:TB],
                        kt_sb[:, pr, hh, kv * P:(kv + 1) * P],
                        qt_sb[:, pr, qs],
                        start=True, stop=True,
                    )
                pt = pp.tile([P, 2, TB], BF16, tag="pt")
                nc.scalar.activation(
                    pt[:, :, q0:TB], sp[:, :, q0:TB],
                    mybir.ActivationFunctionType.Exp,
                    scale=float(SCALE),
                )
                # mask applied post-exp as a 0/1 multiply on the idle GpSimd
                # engine, keeping the DVE queue free for psum evacuation;
                # only the 128-wide diagonal band is ever mixed
                if diag:
                    for hh in (0, 1):
                        nc.gpsimd.tensor_tensor(
                            pt[:, hh, q0:q0 + P],
                            pt[:, hh, q0:q0 + P],
                            mb_sb[:], mybir.AluOpType.mult,
                        )
                elif variant == "general":
                    mg = mgp.tile([P, TB], BF16, tag="mg")
                    nc.sync.dma_start(mg[:], mb[:, kv, qb, :])
                    for hh in (0, 1):
                        nc.gpsimd.tensor_tensor(
                            pt[:, hh, :],
                            pt[:, hh, :],
                            mg[:], mybir.AluOpType.mult,
                        )
                # v-groups are emitted after the block's first scores/exp so
                # they don't delay the softmax chain (AV needs them ~2 steps
                # later)
                if kv == 0 and qb_prologue is not None:
                    qb_prologue(qb)
                # AV runs two steps behind scores so the scores->exp->mask
                # chain (~2us) is fully hidden by two step-times of pipeline
                if len(pendq) >= 2:
                    do_av(*pendq.pop(0))
                pendq.append((qb, kv, q0, pt))
                kvstep += 1
                if early and kvstep >= 4:
                    early.pop(0)()
                elif si < len(spread):
                    want = min(
                        len(spread),
                        (kvstep * len(spread)) // max(1, (9 * n_slots) // 10),
                    )
                    while si < want:
                        spread[si]()
                        si += 1
                else:
                    for _ in range(2):
                        if deferred:
                            deferred.pop(0)()
            # drain the pipeline
            while pendq:
                do_av(*pendq.pop(0))
            while si < len(spread):
                spread[si]()
                si += 1

        # ---- schedule ----------------------------------------------------
        deferred = []
        # xT arrives in per-k-tile chunks over two parallel queues so the
        # projection k-loop (KORD) can chase the transfers; the first weight
        # tiles ride the scalar queue behind only the tiny residents.
        # xT in per-k-tile chunks over two parallel queues (KORD chases the
        # arrival order); first weight tiles ride the scalar queue
        fetch_biases()
        fetch_w(0, nc.scalar)
        for k in range(KT // 2):
            nc.sync.dma_start(xt_sb[:, k, :], xT3[:, k, :])
            nc.gpsimd.dma_start(xt_sb[:, 4 + k, :], xT3[:, 4 + k, :])
        nc.gpsimd.dma_start(wv_sb[:], wvT[:])
        fetch_w(1, nc.scalar)
        fetch_residents()
        for g in proj_filler(0):
            g()

        def v_prologue(qb):
            if variant == "causal":
                tts = range(4 * qb, 4 * qb + 4)
            else:
                tts = range(NTT) if qb == 0 else ()
            for tt in tts:
                emit_v_group(tt)

        attention(0, [], list(proj_filler(1)), qb_prologue=v_prologue)
        fetch_w(2)
        attention(1, list(norm_filler(0)), list(proj_filler(2)))
        fetch_w(3)
        # wo is fetched before attention(2) so it lands well before the
        # first output-projection group (deferred into attention(3))
        nc.sync.dma_start(wo_sb[:], woT[:])
        attention(2, list(norm_filler(1)), list(proj_filler(3)))

        attention(3, list(norm_filler(2)), [])
        while deferred:
            deferred.pop(0)()

    _spill_excess_waits(nc)
    return nc


# ---------------------------------------------------------------------------
# Host side
# ---------------------------------------------------------------------------
_cache: dict[str, bass.Bass] = {}


def _get_program(variant: str) -> bass.Bass:
    if variant not in _cache:
        _cache[variant] = build_program(variant)
    return _cache[variant]


def _mask_variant(mask: np.ndarray) -> str:
    if mask.all():
        return "full"
    if np.array_equal(mask, np.tril(np.ones_like(mask))):
        return "causal"
    return "general"


def _make_in_maps(input, mask, Wq, bq, Wk, bk, Wv, bv, Wo, bo, variant):
    input = np.asarray(input, np.float32)
    mask = np.asarray(mask, bool)
    Wq, Wk, Wv, Wo = (np.asarray(w, np.float32) for w in (Wq, Wk, Wv, Wo))
    bq, bk, bv = (np.asarray(b, np.float32) for b in (bq, bk, bv))
    sel = np.kron(np.eye(8, dtype=np.float32), np.ones((1, DEPTH), np.float32))

    mb_arrs = {}
    if variant != "full":
        # 0/1 multiplicative mask on P = exp(S^T) (applied post-exp)
        maskT01 = mask.T.astype(np.float32)
        if variant == "causal":
            # within-tile triangle: allowed iff q-col >= kv-row
            mb = np.triu(np.ones((P, P), np.float32)).astype(_BF16)
        else:
            mb = (
                maskT01.reshape(NKV, P, NTB, TB)
                .transpose(1, 0, 2, 3)
                .astype(_BF16)
            )
        mb_arrs["mb"] = np.ascontiguousarray(mb)

    in_maps = []
    for c in range(NCORES):
        b, half = c // 2, c % 2
        fs = FH * half
        def tile_kp(wt):
            # [D, F] -> [P, KT, F] with row 128k+p -> [p, k]
            return wt.reshape(KT, P, -1).transpose(1, 0, 2)

        def tile_ft(wt):
            # [D, FH] -> [NFT, P, KT, P]: per f-tile, [p, k, f]
            return np.stack(
                [tile_kp(wt[:, ft * P:(ft + 1) * P]) for ft in range(NFT)]
            )

        m = {
            "xT": np.ascontiguousarray(tile_kp(input[b].T.astype(_BF16))),
            "wqT": np.ascontiguousarray(tile_ft(Wq[fs:fs + FH, :].T.astype(_BF16))),
            "wkT": np.ascontiguousarray(tile_ft(Wk[fs:fs + FH, :].T.astype(_BF16))),
            "wvT": np.ascontiguousarray(tile_kp(Wv[fs:fs + FH, :].T.astype(_BF16))),
            "woT": np.ascontiguousarray(
                Wo[:, fs:fs + FH].T.astype(_BF16).reshape(NFT, P, D).transpose(1, 0, 2)
            ),
            "bq2": np.ascontiguousarray(bq[fs:fs + FH].reshape(NFT, P).T),
            "bk2": np.ascontiguousarray(bk[fs:fs + FH].reshape(NFT, P).T),
            "bvf": np.ascontiguousarray(
                np.broadcast_to(bv[fs:fs + FH], (P, FH)).astype(_BF16)
            ),
            "sel": np.ascontiguousarray(sel.astype(_BF16)),
        }
        m.update(mb_arrs)
        in_maps.append(m)
    return in_maps


def _run(inputs: dict, trace: bool = False, tmpdir=None):
    from concourse.bass_utils import run_bass_kernel_spmd

    variant = _mask_variant(np.asarray(inputs["mask"], bool))
    nc = _get_program(variant)
    in_maps = _make_in_maps(
        inputs["input"], inputs["mask"],
        inputs["Wq"], inputs["bq"], inputs["Wk"], inputs["bk"],
        inputs["Wv"], inputs["bv"], inputs["Wo"], inputs["bo"],
        variant,
    )
    res = run_bass_kernel_spmd(
        nc, in_maps, list(range(NCORES)), trace=trace, tmpdir=tmpdir
    )
    bo = np.asarray(inputs["bo"], np.float32)
    out = np.empty((B, N, D), np.float32)
    for b in range(B):
        yT = (
            res.results[2 * b]["yT"].astype(np.float32)
            + res.results[2 * b + 1]["yT"].astype(np.float32)
        )
        out[b] = yT.T + bo
    return out, res


def kernel(**inputs) -> np.ndarray:
    out, _ = _run(inputs, trace=False)
    return out



# revision 76
# speedup vs baseline: 1.0120x; 1.0120x over previous
"""Multi-head attention (B=4, N=2048, d_model=1024, 16 heads) on 8 trn2 cores.

Sharding: data-parallel over batch (4) x Megatron tensor-parallel over heads
(2-way column-split Wq/Wk/Wv, row-split Wo).  Core c handles batch c//2 and
heads [8*(c%2), 8*(c%2)+8).  Each core emits a partial Y^T [1024, 2048] in
bf16; the host sums core pairs, transposes, and adds the output bias.  No
on-device collectives (a 2-rank AllReduce costs more than the whole compute).

On-device pipeline per core (bf16 matmuls, fp32 PSUM accumulate):
  Q^T [512,2048], zero-padded K^T (each head's 64 rows padded to a full
  128-partition stationary so score LDWEIGHTS background-load instead of
  stalling like row-tiled ones), and V [2048, 8x(64+1)] with a ones column
  so the softmax denominators fall out of the AV matmul.  Per head pair and
  kv-tile: S^T = K_h @ Q_h^T (two serial full-width matmuls), exp on the
  scalar engine with the 1/sqrt(64) scale folded in, then a triu-mask
  multiply on GpSimd for diagonal tiles only — scores/exp/AV all restrict to
  the un-masked column range (q >= kv) of diagonal tiles.  AV (lhsT = V_aug)
  accumulates A^T plus the denominator row; 1/sums are batched bf16
  reciprocals broadcast over the pair's depth rows by a selector matmul.
  Y^T = WoT^T @ A^T, stored bf16.

Scheduling: engines run in-order, so everything is software-pipelined by
emission order.  xT arrives in per-k chunks over two DMA queues and the
projection contraction order chases the arrivals.  AV runs TWO kv-steps
behind scores so the scores->exp->mask chain (~2us) hides entirely.  Next
pair's projections, previous pair's normalization, and the finished
t-blocks of the output projection are paced into the attention loop as PE
filler; the final t-block's output projection is split f=0..2 / f=3 and
interleaved (with warm-keeper matmuls) into the tail reciprocal chain so
the HAM clock gate never re-throttles the PE.
"""

import sys

for _p in ("/opt/trn_rl_repo",):
    if _p not in sys.path:
        sys.path.insert(0, _p)

from contextlib import ExitStack

import ml_dtypes
import numpy as np

import concourse.bass as bass
import concourse.mybir as mybir
import concourse.tile as tile_mod
from concourse.vector_clock import ScopedClock

# ---------------------------------------------------------------------------
# Workaround: this walrus build rejects >1 sync wait on a Drain (CTRL_NO)
# instruction ("Too many sync wait commands").  Tile's end-of-context drain
# carries one wait per live processor, so redistribute the extras onto
# individual EventSemaphore wait instructions.
# ---------------------------------------------------------------------------


def _patched_drain_and_barrier(self, tick_clock, wait_clock):
    nc = self.nc
    drain_inst = nc.sync.drain()
    wait_clock.add_sem_waits(
        drain_inst.ins, ScopedClock({None: tick_clock.global_clock})
    )
    si = drain_inst.ins.sync_info
    waits = list(si.on_wait) if si is not None else []
    if len(waits) > 1:
        assert self.sems is not None
        num2handle = {h.num: h for h in self.sems.allocated().values()}
        drain_inst.ins.sync_info = mybir.SyncInfo(
            on_wait=[waits[0]], on_update=list(si.on_update)
        )
        for w in waits[1:]:
            h = num2handle.get(w.id)
            assert h is not None, f"no sem handle for {w.ant_name} (id {w.id})"
            assert w.wait_mode.startswith("sem-ge"), w.wait_mode
            nc.sync.wait_ge(h, w.wait_value)

    nc.all_engine_barrier()
    assert self.sems is not None
    popped = nc._tile_sem_poison_stack.pop()
    assert popped is self._sem_poison
    nc.clear_and_free_semaphores(list(self.sems.allocated().values()))
    nc.all_engine_barrier()


tile_mod.TileContext._drain_and_barrier = _patched_drain_and_barrier


def _spill_excess_waits(nc: bass.Bass) -> None:
    """This walrus build accepts at most 1 sync wait per instruction (2 for
    EventSemaphore).  Move excess waits onto EventSemaphore instructions
    inserted just before the over-subscribed instruction on the same engine."""
    n_new = 0
    for f in nc.m.functions:
        for blk in f.blocks:
            il = blk.instructions
            out = []
            changed = False
            for inst in il:
                si = inst.sync_info
                waits = list(si.on_wait) if si is not None else []
                cap = 2 if isinstance(inst, mybir.InstEventSemaphore) else 1
                if len(waits) > cap:
                    changed = True
                    extra, keep = waits[:-cap], waits[-cap:]
                    inst.sync_info = mybir.SyncInfo(
                        on_wait=keep, on_update=list(si.on_update)
                    )
                    for j in range(0, len(extra), 2):
                        n_new += 1
                        out.append(
                            mybir.InstEventSemaphore(
                                name=f"{inst.name}-xw{j}",
                                ins=[],
                                outs=[],
                                engine=inst.engine,
                                sync_info=mybir.SyncInfo(
                                    on_wait=extra[j:j + 2], on_update=[]
                                ),
                            )
                        )
                out.append(inst)
            if changed:
                il[:] = out

# ---------------------------------------------------------------------------
# Problem shapes (hardcoded per the task contract).
# ---------------------------------------------------------------------------
B, N, D = 4, 2048, 1024
NHEAD, DEPTH = 16, 64
NCORES = 8
FH = 512          # features per core (8 heads x 64)
HPC = 8           # heads per core
P = 128           # SBUF partitions
TB = 512          # token block (matmul moving free dim)
NTB = N // TB     # 4 token blocks
KT = D // P       # 8 contraction tiles for the projections
NFT = FH // P     # 4 feature tiles (= head pairs)
NTT = N // P      # 16 token tiles
NKV = N // P      # 16 kv tiles
SCALE = 1.0 / np.sqrt(DEPTH)
F32, F32R, BF16 = mybir.dt.float32, mybir.dt.float32r, mybir.dt.bfloat16

_BF16 = ml_dtypes.bfloat16


def build_program(variant: str) -> bass.Bass:
    """variant: 'causal' (tril mask), 'full' (all-true mask), 'general'."""
    assert variant in ("causal", "full", "general")
    nc = bass.Bass()

    # pre-tiled on the host: partition-major layouts for fast (contiguous
    # per-partition) DMA
    xT = nc.declare_dram_parameter("xT", [P, KT, N], BF16, isOutput=False)
    wqT = nc.declare_dram_parameter("wqT", [NFT, P, KT, P], BF16, isOutput=False)
    wkT = nc.declare_dram_parameter("wkT", [NFT, P, KT, P], BF16, isOutput=False)
    wvT = nc.declare_dram_parameter("wvT", [P, KT, FH], BF16, isOutput=False)
    woT = nc.declare_dram_parameter("woT", [P, NFT, D], BF16, isOutput=False)
    bq2 = nc.declare_dram_parameter("bq2", [P, NFT], F32, isOutput=False)
    bk2 = nc.declare_dram_parameter("bk2", [P, NFT], F32, isOutput=False)
    # V bias replicated across partitions: added during the DVE evacuation
    # of the V projection instead of via an extra PE matmul
    bvf = nc.declare_dram_parameter("bvf", [P, FH], BF16, isOutput=False)
    # block-diagonal selector for broadcasting 1/sums rows (bf16)
    sel_d = nc.declare_dram_parameter("sel", [8, 8 * DEPTH], BF16, isOutput=False)
    if variant == "causal":
        # the within-tile triangle: mb[p, c] = 1 iff c >= p
        mb = nc.declare_dram_parameter("mb", [P, P], BF16, isOutput=False)
    elif variant == "general":
        mb = nc.declare_dram_parameter("mb", [P, NKV, NTB, TB], BF16, isOutput=False)
    yT = nc.declare_dram_parameter("yT", [D, N], BF16, isOutput=True)

    def nkv_of(qb):
        return 4 * (qb + 1) if variant == "causal" else NKV

    with tile_mod.TileContext(nc) as tc, ExitStack() as ctx:
        res = ctx.enter_context(tc.tile_pool(name="res", bufs=1))
        wp = ctx.enter_context(tc.tile_pool(name="w", bufs=2))
        pp = ctx.enter_context(tc.tile_pool(name="ppair", bufs=8))
        sums = ctx.enter_context(tc.tile_pool(name="sums", bufs=4))
        yst = ctx.enter_context(tc.tile_pool(name="yst", bufs=4))
        # PSUM: shared accumulator tag (2 banks) + score pair tiles (4) +
        # the two AV accumulators (2) = 8 banks exactly.
        mmp = ctx.enter_context(tc.tile_pool(name="mmp", bufs=2, space="PSUM"))
        pssp = ctx.enter_context(tc.tile_pool(name="pssp", bufs=2, space="PSUM"))
        psav = ctx.enter_context(tc.tile_pool(name="psav", bufs=1, space="PSUM"))
        if variant == "general":
            mgp = ctx.enter_context(tc.tile_pool(name="mg", bufs=4))

        bq_sb = res.tile([P, NFT], F32)
        bk_sb = res.tile([P, NFT], F32)
        bv_sb = res.tile([P, FH], BF16)
        sel_sb = res.tile([8, 8 * DEPTH], BF16)
        mb_sb = None
        if variant == "causal":
            mb_sb = res.tile([P, P], BF16, name="mb_sb")

        def fetch_residents():
            # emitted after the first weight fetches so the scalar queue
            # serves wq0/wk0 first; these are needed a few us later
            nc.scalar.dma_start(bq_sb[:], bq2[:])
            nc.scalar.dma_start(bk_sb[:], bk2[:])
            nc.scalar.dma_start(bv_sb[:], bvf[:])
            nc.scalar.dma_start(sel_sb[:], sel_d[:])
            if variant == "causal":
                nc.scalar.dma_start(mb_sb[:], mb[:])

        qt_sb = res.tile([P, NFT, N], BF16)   # Q^T  [feat, tok]
        # K^T stored zero-padded per head: slot hh holds the head's 64 rows
        # in its own partition half and zeros in the other, so score matmuls
        # are full-width non-tiled (their LDWEIGHTS background-load instead
        # of stalling behind the in-flight matmul like row-tiled ones do).
        kt_sb = res.tile([P, NFT, 2, N], BF16)
        nc.vector.memset(kt_sb[DEPTH:P, :, 0, :], 0.0)
        nc.vector.memset(kt_sb[0:DEPTH, :, 1, :], 0.0)
        v_sb = res.tile([P, NTT, HPC, DEPTH + 1], BF16)  # V + ones col
        nc.gpsimd.memset(v_sb[:, :, :, DEPTH], 1.0)
        a_sb = res.tile([P, NFT, N], BF16)    # A^T (attention output)

        xt_sb = res.tile([P, KT, N], BF16)
        xT3 = xT
        wv_sb = res.tile([P, KT, FH], BF16)

        wq_sbs, wk_sbs = {}, {}

        def fetch_w(ft, eng=None):
            eng = eng or nc.sync
            wq_sbs[ft] = wp.tile([P, KT, P], BF16, tag="wq", name="wq_sb")
            wk_sbs[ft] = wp.tile([P, KT, P], BF16, tag="wk", name="wk_sb")
            eng.dma_start(wq_sbs[ft][:], wqT[ft])
            eng.dma_start(wk_sbs[ft][:], wkT[ft])

        # contraction order chases the startup DMA arrival order (sync queue
        # carries k0-3, gpsimd k4-7, both draining in parallel)
        KORD = (0, 4, 1, 5, 2, 6, 3, 7)

        def emit_qk_group(ft, tb, which):
            ts = slice(tb * TB, (tb + 1) * TB)
            w_sb = wq_sbs[ft] if which == "q" else wk_sbs[ft]
            dst = qt_sb if which == "q" else kt_sb
            bias = bq_sb if which == "q" else bk_sb
            ps = mmp.tile([P, TB], F32, tag="acc", name="pqk")
            for j, k in enumerate(KORD):
                nc.tensor.matmul(
                    ps[:], w_sb[:, k, :], xt_sb[:, k, ts],
                    start=(j == 0), stop=(j == KT - 1),
                )
            if which == "k":
                for hh in (0, 1):
                    rows = slice(DEPTH * hh, DEPTH * hh + DEPTH)
                    nc.vector.tensor_tensor(
                        kt_sb[rows, ft, hh, ts], ps[rows],
                        bias[rows, ft, None].to_broadcast((DEPTH, TB)),
                        mybir.AluOpType.add,
                    )
            else:
                nc.vector.tensor_tensor(
                    dst[:, ft, ts], ps[:],
                    bias[:, ft, None].to_broadcast((P, TB)),
                    mybir.AluOpType.add,
                )

        def proj_filler(ft):
            for tb in range(NTB):
                for which in ("q", "k"):
                    yield lambda ft=ft, tb=tb, which=which: emit_qk_group(
                        ft, tb, which
                    )

        def emit_v_group(tt):
            pv = mmp.tile([P, TB], F32, tag="acc", name="pv")
            for j, k in enumerate(KORD):
                nc.tensor.matmul(
                    pv[:], xt_sb[:, k, tt * P:(tt + 1) * P], wv_sb[:, k, :],
                    start=(j == 0), stop=(j == KT - 1),
                )
            nc.vector.tensor_tensor(
                v_sb[:, tt, :, 0:DEPTH],
                pv[:].rearrange("p (h d) -> p h d", h=HPC),
                bv_sb[:].rearrange("p (h d) -> p h d", h=HPC),
                mybir.AluOpType.add,
            )

        sums_ps = {}
        rall_store = {}

        def emit_norm_qb(pr, rall, row0, qb, tail=False):
            """bc = broadcast of 1/sums rows (row0, row0+1) over the head
            depth; one in-place multiply normalizes both heads of the pair.
            The tail variant borrows a retired AV psum bank so the held
            output-projection partials can't deadlock the acc ring."""
            qs = slice(qb * TB, (qb + 1) * TB)
            nrows = rall.shape[0]
            if tail:
                bc = psav.tile([P, TB], F32, tag="av0", name="bc")
            else:
                bc = mmp.tile([P, TB], F32, tag="acc", name="bc")
            nc.tensor.matmul(
                bc[:],
                sel_sb[0:nrows, row0 * DEPTH:(row0 + 2) * DEPTH],
                rall[:],
                start=True, stop=True,
            )
            nc.vector.tensor_tensor(
                a_sb[:, pr, qs], a_sb[:, pr, qs], bc[:],
                mybir.AluOpType.mult,
            )

        def norm_filler(pr):
            def recip(pr=pr, part=0):
                if part == 0:
                    rall_store[pr] = sums.tile([8, TB], BF16, tag="rall", name="rall")
                with nc.allow_low_precision(
                    reason="bf16 1/sum scales each softmax row uniformly"
                ):
                    cs = slice(part * P, (part + 1) * P)
                    nc.vector.reciprocal(
                        rall_store[pr][:, cs], sums_ps[pr][:, cs]
                    )

            for part in range(TB // P):
                yield lambda pr=pr, part=part: recip(pr, part)
            for qb in range(NTB):
                yield lambda pr=pr, qb=qb: emit_norm_qb(
                    pr, rall_store[pr], 2 * qb, qb
                )

        wo_sb = res.tile([P, NFT, D], BF16)
        yT3 = yT.rearrange("(o p) t -> p o t", p=P)

        def outproj_pre(ot, tb):
            """f=0..2 partial accumulation — consumable before the final
            pr's normalization lands, keeping the PE warm through the
            tail reciprocal chain."""
            ts = slice(tb * TB, (tb + 1) * TB)
            py = mmp.tile([P, TB], F32, tag="acc", name="py")
            for f in range(NFT - 1):
                nc.tensor.matmul(
                    py[:], wo_sb[:, f, ot * P:(ot + 1) * P],
                    a_sb[:, f, ts],
                    start=(f == 0), stop=False,
                )
            return py

        def outproj_fin(py, ot, tb):
            ts = slice(tb * TB, (tb + 1) * TB)
            nc.tensor.matmul(
                py[:], wo_sb[:, NFT - 1, ot * P:(ot + 1) * P],
                a_sb[:, NFT - 1, ts],
                start=False, stop=True,
            )
            yt = yst.tile([P, TB], BF16, tag="yt")
            nc.vector.tensor_copy(yt[:], py[:])
            # spread output stores over DMA queues; the scalar (exp) queue
            # only joins for the final q-block, after the last activation
            if tb == NTB - 1:
                eng = (nc.sync, nc.gpsimd, nc.scalar)[ot % 3]
            else:
                eng = nc.sync if ot % 2 == 0 else nc.gpsimd
            eng.dma_start(yT3[:, ot, ts], yt[:])

        def emit_outproj_group(ot, tb):
            outproj_fin(outproj_pre(ot, tb), ot, tb)

        def attention(pr, early, spread, qb_prologue=None):
            """early: fillers consumed one per kv-step (from step 4);
            spread: fillers paced over ~70% of the kv-steps.  Scores run one
            step ahead of AV across qb boundaries so the PE never drains."""
            last = pr == NFT - 1
            if not last:
                sums_p = sums.tile([8, TB], F32, tag="sums_p", name="sums_p")
                sums_ps[pr] = sums_p
            steps = [(qb, kv) for qb in range(NTB) for kv in range(nkv_of(qb))]
            n_slots = len(steps)
            si = 0
            kvstep = 0
            avs = {}
            s3s = {}

            def qb_end(qb):
                qs = slice(qb * TB, (qb + 1) * TB)
                av = avs.pop(qb)
                # sum-row copies first: they gate the reciprocal chain,
                # while the bulk A^T copies only gate the norm multiply
                for hh in (0, 1):
                    srow = sums.tile([P, TB], F32, tag="srow", name="srow")
                    nc.vector.tensor_copy(
                        srow[DEPTH:DEPTH + 1, :], av[hh][DEPTH:DEPTH + 1, :]
                    )
                    if last:
                        nc.gpsimd.dma_start(
                            s3s[qb][hh:hh + 1, :], srow[DEPTH:DEPTH + 1, :]
                        )
                    else:
                        nc.gpsimd.dma_start(
                            sums_p[2 * qb + hh:2 * qb + hh + 1, :],
                            srow[DEPTH:DEPTH + 1, :],
                        )
                for hh in (0, 1):
                    nc.vector.tensor_copy(
                        a_sb[64 * hh:64 * hh + 64, pr, qs], av[hh][0:DEPTH, :]
                    )
                if last:
                    s3 = s3s.pop(qb)

                    rall3 = [None]

                    def norm3_part(part, qb=qb, s3=s3, rall3=rall3):
                        if part == 0:
                            rall3[0] = sums.tile([2, TB], BF16, tag="r3", name="rall3")
                        with nc.allow_low_precision(
                            reason="bf16 1/sum scales each softmax row uniformly"
                        ):
                            cs = slice(part * P, (part + 1) * P)
                            nc.vector.reciprocal(rall3[0][:, cs], s3[:, cs])
                        if part == TB // P - 1:
                            emit_norm_qb(pr, rall3[0], 0, qb, tail=(qb == NTB - 1))

                    if qb == NTB - 1:
                        # final q-block: interleave the f=0..2 output-proj
                        # partials (valid pre-norm) with the reciprocal
                        # chain so the PE never idles into a HAM rethrottle,
                        # then finish each column group after the norm
                        pys = {}

                        def pre(ot, qb=qb):
                            pys[ot] = outproj_pre(ot, qb)

                        def fin(ot, qb=qb):
                            outproj_fin(pys.pop(ot), ot, qb)

                        def warmer(qb=qb):
                            # keeps the PE streaming through the reciprocal
                            # chain so HAM doesn't rethrottle before the
                            # final output-projection burst; output unused
                            dj = psav.tile([P, TB], F32, tag="av1", name="dj")
                            ts_ = slice(qb * TB, (qb + 1) * TB)
                            for f in range(NFT - 1):
                                nc.tensor.matmul(
                                    dj[:], wo_sb[:, f, 0:P], a_sb[:, f, ts_],
                                    start=(f == 0), stop=(f == NFT - 2),
                                )

                        deferred.append(lambda: pre(0))
                        deferred.append(lambda: pre(1))
                        deferred.append(lambda: norm3_part(0))
                        deferred.append(lambda: norm3_part(1))
                        deferred.append(warmer)
                        deferred.append(lambda: norm3_part(2))
                        deferred.append(warmer)
                        deferred.append(lambda: norm3_part(3))
                        deferred.append(warmer)
                        for ot in range(D // P):
                            deferred.append(lambda ot=ot: fin(ot))
                            if ot + 2 < D // P:
                                deferred.append(lambda ot=ot: pre(ot + 2))
                    else:
                        for part in range(TB // P):
                            deferred.append(lambda part=part: norm3_part(part))
                        for ot in range(D // P):
                            deferred.append(
                                lambda ot=ot, tb=qb: emit_outproj_group(ot, tb)
                            )

            def do_av(pqb, pkv, pq0, ppt):
                if pkv == 0:
                    avs[pqb] = [
                        psav.tile([P, TB], F32, tag=f"av{h}", name=f"av{h}")
                        for h in (0, 1)
                    ]
                for hh in (0, 1):
                    nc.tensor.matmul(
                        avs[pqb][hh][0:DEPTH + 1, pq0:TB],
                        v_sb[:, pkv, 2 * pr + hh, :],
                        ppt[:, hh, pq0:TB],
                        start=(pkv == 0), stop=(pkv == nkv_of(pqb) - 1),
                    )
                if pkv == nkv_of(pqb) - 1:
                    qb_end(pqb)

            pendq = []
            for qb, kv in steps:
                if kv == 0 and last:
                    s3s[qb] = sums.tile([2, TB], F32, tag="s3", name="s3")
                # diagonal kv-tiles only need q >= kv: columns < q0 of this
                # q-block are fully masked, so scores/exp/mask/AV all skip
                # them (the per-tile mask reduces to one triu pattern).
                diag = variant == "causal" and kv >= 4 * qb
                q0 = (kv - 4 * qb) * P if diag else 0
                qs = slice(qb * TB + q0, (qb + 1) * TB)
                sp = pssp.tile([P, 2, TB], F32, tag="sp")
                for hh in (0, 1):
                    nc.tensor.matmul(
                        sp[:, hh, q0:TB],
                        kt_sb[:, pr, hh, kv * P:(kv + 1) * P],
                        qt_sb[:, pr, qs],
                        start=True, stop=True,
                    )
                pt = pp.tile([P, 2, TB], BF16, tag="pt")
                nc.scalar.activation(
                    pt[:, :, q0:TB], sp[:, :, q0:TB],
                    mybir.ActivationFunctionType.Exp,
                    scale=float(SCALE),
                )
                # mask applied post-exp as a 0/1 multiply on the idle GpSimd
                # engine, keeping the DVE queue free for psum evacuation;
                # only the 128-wide diagonal band is ever mixed
                if diag:
                    for hh in (0, 1):
                        nc.gpsimd.tensor_tensor(
                            pt[:, hh, q0:q0 + P],
                            pt[:, hh, q0:q0 + P],
                            mb_sb[:], mybir.AluOpType.mult,
                        )
                elif variant == "general":
                    mg = mgp.tile([P, TB], BF16, tag="mg")
                    nc.sync.dma_start(mg[:], mb[:, kv, qb, :])
                    for hh in (0, 1):
                        nc.gpsimd.tensor_tensor(
                            pt[:, hh, :],
                            pt[:, hh, :],
                            mg[:], mybir.AluOpType.mult,
                        )
                # v-groups are emitted after the block's first scores/exp so
                # they don't delay the softmax chain (AV needs them ~2 steps
                # later)
                if kv == 0 and qb_prologue is not None:
                    qb_prologue(qb)
                # AV runs two steps behind scores so the scores->exp->mask
                # chain (~2us) is fully hidden by two step-times of pipeline
                if len(pendq) >= 2:
                    do_av(*pendq.pop(0))
                pendq.append((qb, kv, q0, pt))
                kvstep += 1
                if early and kvstep >= 4:
                    early.pop(0)()
                elif si < len(spread):
                    want = min(
                        len(spread),
                        (kvstep * len(spread)) // max(1, (9 * n_slots) // 10),
                    )
                    while si < want:
                        spread[si]()
                        si += 1
                else:
                    for _ in range(2):
                        if deferred:
                            deferred.pop(0)()
            # drain the pipeline
            while pendq:
                do_av(*pendq.pop(0))
            while si < len(spread):
                spread[si]()
                si += 1

        # ---- schedule ----------------------------------------------------
        deferred = []
        # xT arrives in per-k-tile chunks over two parallel queues so the
        # projection k-loop (KORD) can chase the transfers; the first weight
        # tiles ride the scalar queue behind only the tiny residents.
        # xT in per-k-tile chunks over two parallel queues (KORD chases the
        # arrival order); first weight tiles ride the scalar queue
        fetch_w(0, nc.scalar)
        for k in range(KT // 2):
            nc.sync.dma_start(xt_sb[:, k, :], xT3[:, k, :])
            nc.gpsimd.dma_start(xt_sb[:, 4 + k, :], xT3[:, 4 + k, :])
        nc.gpsimd.dma_start(wv_sb[:], wvT[:])
        fetch_w(1, nc.scalar)
        fetch_residents()
        for g in proj_filler(0):
            g()

        def v_prologue(qb):
            if variant == "causal":
                tts = range(4 * qb, 4 * qb + 4)
            else:
                tts = range(NTT) if qb == 0 else ()
            for tt in tts:
                emit_v_group(tt)

        attention(0, [], list(proj_filler(1)), qb_prologue=v_prologue)
        fetch_w(2)
        attention(1, list(norm_filler(0)), list(proj_filler(2)))
        fetch_w(3)
        # wo is fetched before attention(2) so it lands well before the
        # first output-projection group (deferred into attention(3))
        nc.sync.dma_start(wo_sb[:], woT[:])
        attention(2, list(norm_filler(1)), list(proj_filler(3)))

        attention(3, list(norm_filler(2)), [])
        while deferred:
            deferred.pop(0)()

    _spill_excess_waits(nc)
    return nc


# ---------------------------------------------------------------------------
# Host side
# ---------------------------------------------------------------------------
_cache: dict[str, bass.Bass] = {}


def _get_program(variant: str) -> bass.Bass:
    if variant not in _cache:
        _cache[variant] = build_program(variant)
    return _cache[variant]


def _mask_variant(mask: np.ndarray) -> str:
    if mask.all():
        return "full"
    if np.array_equal(mask, np.tril(np.ones_like(mask))):
        return "causal"
    return "general"


def _make_in_maps(input, mask, Wq, bq, Wk, bk, Wv, bv, Wo, bo, variant):
    input = np.asarray(input, np.float32)
    mask = np.asarray(mask, bool)
    Wq, Wk, Wv, Wo = (np.asarray(w, np.float32) for w in (Wq, Wk, Wv, Wo))
    bq, bk, bv = (np.asarray(b, np.float32) for b in (bq, bk, bv))
    sel = np.kron(np.eye(8, dtype=np.float32), np.ones((1, DEPTH), np.float32))

    mb_arrs = {}
    if variant != "full":
        # 0/1 multiplicative mask on P = exp(S^T) (applied post-exp)
        maskT01 = mask.T.astype(np.float32)
        if variant == "causal":
            # within-tile triangle: allowed iff q-col >= kv-row
            mb = np.triu(np.ones((P, P), np.float32)).astype(_BF16)
        else:
            mb = (
                maskT01.reshape(NKV, P, NTB, TB)
                .transpose(1, 0, 2, 3)
                .astype(_BF16)
            )
        mb_arrs["mb"] = np.ascontiguousarray(mb)

    in_maps = []
    for c in range(NCORES):
        b, half = c // 2, c % 2
        fs = FH * half
        def tile_kp(wt):
            # [D, F] -> [P, KT, F] with row 128k+p -> [p, k]
            return wt.reshape(KT, P, -1).transpose(1, 0, 2)

        def tile_ft(wt):
            # [D, FH] -> [NFT, P, KT, P]: per f-tile, [p, k, f]
            return np.stack(
                [tile_kp(wt[:, ft * P:(ft + 1) * P]) for ft in range(NFT)]
            )

        m = {
            "xT": np.ascontiguousarray(tile_kp(input[b].T.astype(_BF16))),
            "wqT": np.ascontiguousarray(tile_ft(Wq[fs:fs + FH, :].T.astype(_BF16))),
            "wkT": np.ascontiguousarray(tile_ft(Wk[fs:fs + FH, :].T.astype(_BF16))),
            "wvT": np.ascontiguousarray(tile_kp(Wv[fs:fs + FH, :].T.astype(_BF16))),
            "woT": np.ascontiguousarray(
                Wo[:, fs:fs + FH].T.astype(_BF16).reshape(NFT, P, D).transpose(1, 0, 2)
            ),
            "bq2": np.ascontiguousarray(bq[fs:fs + FH].reshape(NFT, P).T),
            "bk2": np.ascontiguousarray(bk[fs:fs + FH].reshape(NFT, P).T),
            "bvf": np.ascontiguousarray(
                np.broadcast_to(bv[fs:fs + FH], (P, FH)).astype(_BF16)
            ),
            "sel": np.ascontiguousarray(sel.astype(_BF16)),
        }
        m.update(mb_arrs)
        in_maps.append(m)
    return in_maps


def _run(inputs: dict, trace: bool = False, tmpdir=None):
    from concourse.bass_utils import run_bass_kernel_spmd

    variant = _mask_variant(np.asarray(inputs["mask"], bool))
    nc = _get_program(variant)
    in_maps = _make_in_maps(
        inputs["input"], inputs["mask"],
        inputs["Wq"], inputs["bq"], inputs["Wk"], inputs["bk"],
        inputs["Wv"], inputs["bv"], inputs["Wo"], inputs["bo"],
        variant,
    )
    res = run_bass_kernel_spmd(
        nc, in_maps, list(range(NCORES)), trace=trace, tmpdir=tmpdir
    )
    bo = np.asarray(inputs["bo"], np.float32)
    out = np.empty((B, N, D), np.float32)
    for b in range(B):
        yT = (
            res.results[2 * b]["yT"].astype(np.float32)
            + res.results[2 * b + 1]["yT"].astype(np.float32)
        )
        out[b] = yT.T + bo
    return out, res


def kernel(**inputs) -> np.ndarray:
    out, _ = _run(inputs, trace=False)
    return out

